# revision 10
# baseline (speedup 1.0000x reference)
"""Trainium2 Bass kernel for DigitConvolutionalModel.

Computes, for x [32768, 784] viewed as 28x28 images:
    feat = relu(conv3x3_valid(x))      # [B, 676]
    out  = feat @ W + b                # [B, 10]

Strategy (pure data parallel over 8 cores, 4096 rows each):
  - Host pre-transposes each core's shard to pixel-major, chunk-major
    layout xh [8 chunks, 784 pixels, 512 batch] so the contraction dims
    sit on SBUF partitions (TensorE contracts partitions only) and each
    chunk loads with big strided DMAs.
  - Loads cast fp32 -> bf16 in the DMA datapath (SWDGE); HBM still reads
    the full fp32 input, SBUF holds bf16.
  - The 3x3 conv is a banded matmul y^T = C^T @ x^T using two constant
    blocks C1/C2 [112, 104] built on host from conv_w: input rows are
    tiled 4 image rows (112 pixels) per partition group, output rows
    4 conv rows (104 pixels) per PSUM tile.
  - ReLU evacuates PSUM -> SBUF bf16 (split between ScalarE and VectorE).
  - The 676->10 linear layer contracts the same pixel tiles against
    host-packed W blocks, accumulating out^T [10, 512] fp32 in PSUM;
    bias is added during the PSUM->SBUF copy.
  - Device emits out^T [10, 4096]; host transposes back.

Walrus accepts only ONE semaphore wait per engine instruction, so the
kernel is arranged so every instruction needs at most one: constants are
pre-touched by tiny warm-up ops, each chunk's x-tile DMA is absorbed by
a touch matmul, and redundant same-engine waits Tile emits are stripped.
"""

import numpy as np

try:
    from concourse import bass, mybir
    from concourse.tile import TileContext
    from concourse.bass_utils import run_bass_kernel_spmd
except ImportError:  # path used when concourse is not already importable
    import sys

    sys.path.insert(0, "/opt/trn_rl_repo")
    from concourse import bass, mybir
    from concourse.tile import TileContext
    from concourse.bass_utils import run_bass_kernel_spmd

from concourse.vector_clock import ScopedClock


def _patched_drain_and_barrier(self, tick_clock, wait_clock):
    """Replacement for TileContext._drain_and_barrier: walrus rejects
    instructions carrying more than one sync wait, but the kernel-tail
    drain aggregates a wait per logical proc (~14 here). Emit a chain of
    single-wait drains on the sync queue instead."""
    nc = self.nc
    drain_inst = nc.sync.drain()
    wait_clock.add_sem_waits(
        drain_inst.ins, ScopedClock({None: tick_clock.global_clock})
    )
    si = drain_inst.ins.sync_info
    waits = list(si.on_wait or []) if si else []
    if len(waits) > 1:
        drain_inst.ins.sync_info = mybir.SyncInfo(
            on_wait=waits[:1], on_update=si.on_update
        )
        for w in waits[1:]:
            extra = nc.sync.drain()
            esi = extra.ins.sync_info
            extra.ins.sync_info = mybir.SyncInfo(
                on_wait=[w], on_update=(esi.on_update if esi else [])
            )
    nc.all_engine_barrier()
    popped = nc._tile_sem_poison_stack.pop()
    assert popped is self._sem_poison
    nc.clear_and_free_semaphores(list(self.sems.allocated().values()))
    nc.all_engine_barrier()


TileContext._drain_and_barrier = _patched_drain_and_barrier

N_CORES = 8
B = 32768
B_CORE = B // N_CORES  # 4096
CHUNK = 512
N_CHUNKS = B_CORE // CHUNK  # 8
NT = 7  # pixel-group tiles of 4 image rows (112 pixels); 7*4 = 28 rows

F32 = mybir.dt.float32
BF16 = mybir.dt.bfloat16
RELU = mybir.ActivationFunctionType.Relu
IDENT = mybir.ActivationFunctionType.Identity

_NC_CACHE = {}


def _build_nc():
    nc = bass.Bass(
        "TRN2", target_bir_lowering=False, debug=False, num_devices=N_CORES
    )

    # chunk-major pixel-major input: row 784*n + 112*t + p, col = batch
    xh = nc.dram_tensor("xh", [N_CHUNKS * 784, CHUNK], F32, kind="ExternalInput")
    c1_d = nc.dram_tensor("c1", [112, 104], BF16, kind="ExternalInput")
    c2_d = nc.dram_tensor("c2", [112, 104], BF16, kind="ExternalInput")
    wp_d = nc.dram_tensor("wp", [104, 70], BF16, kind="ExternalInput")
    bias_d = nc.dram_tensor("bias_in", [10, 1], F32, kind="ExternalInput")
    out_t = nc.dram_tensor("out_t", [10, B_CORE], F32, kind="ExternalOutput")

    with TileContext(nc) as tc:
        with (
            tc.tile_pool(name="const", bufs=1) as cpool,
            tc.tile_pool(name="xc", bufs=1) as xpool,
            tc.tile_pool(name="ry_a", bufs=4) as rypool_a,
            tc.tile_pool(name="ry_v", bufs=4) as rypool_v,
            tc.tile_pool(name="outT", bufs=1) as opool,
            tc.tile_pool(name="yps_a", bufs=2, space="PSUM") as ypool_a,
            tc.tile_pool(name="yps_v", bufs=2, space="PSUM") as ypool_v,
            tc.tile_pool(name="warmp", bufs=1, space="PSUM") as warmpool,
            tc.tile_pool(name="opsum", bufs=2, space="PSUM") as opsum,
        ):
            c1_sb = cpool.tile([112, 104], BF16, tag="c1")
            nc.sync.dma_start(c1_sb[:], c1_d.ap())
            c2_sb = cpool.tile([112, 104], BF16, tag="c2")
            nc.sync.dma_start(c2_sb[:], c2_d.ap())
            wp_sb = cpool.tile([104, 70], BF16, tag="wp")
            nc.sync.dma_start(wp_sb[:], wp_d.ap())
            bias_sb = cpool.tile([10, 1], F32, tag="bias")
            nc.sync.dma_start(bias_sb[:], bias_d.ap())

            outT_sb = opool.tile([10, B_CORE], F32, tag="outT")

            # Pre-touch each constant with a tiny op so real instructions'
            # dependency on its DMA is satisfied by engine program order
            # (walrus allows a single sync wait per instruction).
            warm = warmpool.tile([4, 4], F32, tag="warm")
            nc.tensor.matmul(warm[:], c1_sb[:, 0:4], c1_sb[:, 0:4])
            nc.tensor.matmul(warm[:], c2_sb[:, 0:4], c2_sb[:, 0:4])
            nc.tensor.matmul(warm[:], wp_sb[:, 0:4], wp_sb[:, 0:4])
            warm_act = cpool.tile([10, 1], F32, tag="warm_act")
            nc.scalar.activation(warm_act[:], bias_sb[:], IDENT, bias=bias_sb[:])

            # x chunk tiles [112, 7*512] bf16: block t at cols 512t..512t+511
            # holds pixels 112t..112t+111. Loaded with 2 strided cast-DMAs
            # (SWDGE) per chunk from the fp32 chunk-major DRAM image.
            xc = []
            for n in range(N_CHUNKS):
                tile = xpool.tile([112, NT * CHUNK], BF16, tag=f"xc{n}")
                for lo, hi in ((0, 4), (4, 7)):
                    src = bass.AP(
                        xh,
                        (784 * n + 112 * lo) * CHUNK,
                        [[CHUNK, 112], [112 * CHUNK, hi - lo], [1, CHUNK]],
                    )
                    nc.gpsimd.dma_start(
                        tile[:, CHUNK * lo : CHUNK * hi], src
                    )
                xc.append(tile)

            for n in range(N_CHUNKS):
                # Touch matmul: absorbs this chunk's DMA wait on PE so the
                # conv matmuls only carry their PSUM-slot wait.
                nc.tensor.matmul(warm[:], xc[n][:, 0:4], xc[n][:, 0:4])
                nc.tensor.matmul(
                    warm[:],
                    xc[n][:, 6 * CHUNK : 6 * CHUNK + 4],
                    xc[n][:, 6 * CHUNK : 6 * CHUNK + 4],
                )
                rys = []
                for t in range(NT):
                    m = 104 if t < 6 else 52
                    on_act = t % 2 == 0
                    yps = (ypool_a if on_act else ypool_v).tile(
                        [m, CHUNK], F32, tag="yps"
                    )
                    nc.tensor.matmul(
                        yps[:],
                        c1_sb[:, 0:m],
                        xc[n][:, CHUNK * t : CHUNK * (t + 1)],
                        start=True,
                        stop=(t == 6),
                    )
                    if t < 6:
                        nc.tensor.matmul(
                            yps[:],
                            c2_sb[:],
                            xc[n][:, CHUNK * (t + 1) : CHUNK * (t + 2)],
                            start=False,
                            stop=True,
                        )
                    ry = (rypool_a if on_act else rypool_v).tile(
                        [m, CHUNK], BF16, tag="ry"
                    )
                    if on_act:
                        nc.scalar.activation(ry[:], yps[:], RELU)
                    else:
                        nc.vector.tensor_relu(ry[:], yps[:])
                    rys.append(ry)

                ops = opsum.tile([10, CHUNK], F32, tag="ops")
                for t in range(NT):
                    k = 104 if t < 6 else 52
                    nc.tensor.matmul(
                        ops[:],
                        wp_sb[0:k, 10 * t : 10 * (t + 1)],
                        rys[t][:],
                        start=(t == 0),
                        stop=(t == 6),
                    )
                nc.scalar.activation(
                    outT_sb[:, CHUNK * n : CHUNK * (n + 1)],
                    ops[:],
                    IDENT,
                    bias=bias_sb[:],
                )

            # SP queue is free of bulk loads now; its DMA lanes are fresh,
            # so this trigger carries only the data wait.
            nc.sync.dma_start(out_t.ap(), outT_sb[:])

    _strip_self_waits(nc)
    return nc


_ENGINE_SEM_PREFIX = {
    mybir.EngineType.PE: "PE_",
    mybir.EngineType.Activation: "Activation_",
    mybir.EngineType.DVE: "DVE_",
    mybir.EngineType.Pool: "Pool_",
    mybir.EngineType.SP: "SP_",
}


def _strip_self_waits(nc):
    """Drop semaphore waits an instruction holds on its OWN engine's
    completion counter. Engines execute their queue strictly in order, so
    a wait on the own-engine sem at a value covered by program order is
    redundant — but Tile still emits it, and walrus rejects compute
    instructions carrying more than one sync wait."""
    for fn in nc.m.functions:
        for blk in fn.blocks:
            for inst in blk.instructions:
                tn = type(inst).__name__
                if tn in ("InstDrain", "InstEventSemaphore", "InstDMACopy"):
                    continue
                si = inst.sync_info
                if si is None or not si.on_wait or len(si.on_wait) < 2:
                    continue
                pref = _ENGINE_SEM_PREFIX.get(inst.engine)
                if pref is None:
                    continue
                kept = [w for w in si.on_wait if not w.ant_name.startswith(pref)]
                if len(kept) != len(si.on_wait):
                    inst.sync_info = mybir.SyncInfo(
                        on_wait=kept, on_update=si.on_update
                    )


def _build_consts(conv_w, W, b):
    conv_w = np.asarray(conv_w, np.float32)
    W = np.asarray(W, np.float32)
    b = np.asarray(b, np.float32)

    # C1: input rows 4t+rl (rl 0..3) -> output conv rows 4t+il (il 0..3)
    # C2: input rows 4(t+1)+rl      -> output conv rows 4t+il
    c1 = np.zeros((112, 104), np.float32)
    c2 = np.zeros((112, 104), np.float32)
    for rl in range(4):
        for c in range(28):
            for il in range(4):
                for j in range(26):
                    dj = c - j
                    if not (0 <= dj <= 2):
                        continue
                    di = rl - il
                    if 0 <= di <= 2:
                        c1[rl * 28 + c, il * 26 + j] = conv_w[di, dj]
                    di2 = 4 + rl - il
                    if 0 <= di2 <= 2:
                        c2[rl * 28 + c, il * 26 + j] = conv_w[di2, dj]

    # W packed: block t holds rows for conv-output rows 4t..4t+3
    wp = np.zeros((104, 70), np.float32)
    for t in range(6):
        wp[:, 10 * t : 10 * (t + 1)] = W[104 * t : 104 * (t + 1)]
    wp[0:52, 60:70] = W[624:676]

    import ml_dtypes

    bf16 = ml_dtypes.bfloat16
    return (
        c1.astype(bf16),
        c2.astype(bf16),
        wp.astype(bf16),
        b.reshape(10, 1).copy(),
    )


def _run(inputs, trace=False):
    x = np.asarray(inputs["x"], np.float32)
    conv_w = inputs["conv_w"]
    W = inputs["W"]
    b = inputs["b"]

    if "nc" not in _NC_CACHE:
        _NC_CACHE["nc"] = _build_nc()
    nc = _NC_CACHE["nc"]

    c1, c2, wp, bias = _build_consts(conv_w, W, b)

    in_maps = []
    for c in range(N_CORES):
        shard = x[c * B_CORE : (c + 1) * B_CORE]  # [4096, 784]
        # [8, 512, 784] -> [8, 784, 512] -> rows (n, pixel), cols batch
        xh = np.ascontiguousarray(
            shard.reshape(N_CHUNKS, CHUNK, 784).transpose(0, 2, 1)
        ).reshape(N_CHUNKS * 784, CHUNK)
        in_maps.append(
            {"xh": xh, "c1": c1, "c2": c2, "wp": wp, "bias_in": bias}
        )

    res = run_bass_kernel_spmd(
        nc, in_maps, core_ids=list(range(N_CORES)), trace=trace
    )
    out = np.concatenate(
        [np.asarray(res.results[c]["out_t"]).T for c in range(N_CORES)], axis=0
    )
    return out, res


def kernel(**inputs) -> np.ndarray:
    return _run(inputs, trace=False)[0]


# revision 11
# speedup vs baseline: 1.0020x; 1.0020x over previous
"""Trainium2 Bass kernel for DigitConvolutionalModel.

Computes, for x [32768, 784] viewed as 28x28 images:
    feat = relu(conv3x3_valid(x))      # [B, 676]
    out  = feat @ W + b                # [B, 10]

Strategy (pure data parallel over 8 cores, 4096 rows each):
  - Host pre-transposes each core's shard to pixel-major, chunk-major
    layout xh [8 chunks, 784 pixels, 512 batch] so the contraction dims
    sit on SBUF partitions (TensorE contracts partitions only) and each
    chunk loads with big strided DMAs.
  - Loads cast fp32 -> bf16 in the DMA datapath (SWDGE); HBM still reads
    the full fp32 input, SBUF holds bf16.
  - The 3x3 conv is a banded matmul y^T = C^T @ x^T using two constant
    blocks C1/C2 [112, 104] built on host from conv_w: input rows are
    tiled 4 image rows (112 pixels) per partition group, output rows
    4 conv rows (104 pixels) per PSUM tile.
  - ReLU evacuates PSUM -> SBUF bf16 (split between ScalarE and VectorE).
  - The 676->10 linear layer contracts the same pixel tiles against
    host-packed W blocks, accumulating out^T [10, 512] fp32 in PSUM;
    bias is added during the PSUM->SBUF copy.
  - Device emits out^T [10, 4096]; host transposes back.

Walrus accepts only ONE semaphore wait per engine instruction, so the
kernel is arranged so every instruction needs at most one: constants are
pre-touched by tiny warm-up ops, each chunk's x-tile DMA is absorbed by
a touch matmul, and redundant same-engine waits Tile emits are stripped.
"""

import numpy as np

try:
    from concourse import bass, mybir
    from concourse.tile import TileContext
    from concourse.bass_utils import run_bass_kernel_spmd
except ImportError:  # path used when concourse is not already importable
    import sys

    sys.path.insert(0, "/opt/trn_rl_repo")
    from concourse import bass, mybir
    from concourse.tile import TileContext
    from concourse.bass_utils import run_bass_kernel_spmd

from concourse.vector_clock import ScopedClock


def _patched_drain_and_barrier(self, tick_clock, wait_clock):
    """Replacement for TileContext._drain_and_barrier: walrus rejects
    instructions carrying more than one sync wait, but the kernel-tail
    drain aggregates a wait per logical proc (~14 here). Emit a chain of
    single-wait drains on the sync queue instead."""
    nc = self.nc
    drain_inst = nc.sync.drain()
    wait_clock.add_sem_waits(
        drain_inst.ins, ScopedClock({None: tick_clock.global_clock})
    )
    si = drain_inst.ins.sync_info
    waits = list(si.on_wait or []) if si else []
    if len(waits) > 1:
        drain_inst.ins.sync_info = mybir.SyncInfo(
            on_wait=waits[:1], on_update=si.on_update
        )
        for w in waits[1:]:
            extra = nc.sync.drain()
            esi = extra.ins.sync_info
            extra.ins.sync_info = mybir.SyncInfo(
                on_wait=[w], on_update=(esi.on_update if esi else [])
            )
    nc.all_engine_barrier()
    popped = nc._tile_sem_poison_stack.pop()
    assert popped is self._sem_poison
    nc.clear_and_free_semaphores(list(self.sems.allocated().values()))
    nc.all_engine_barrier()


TileContext._drain_and_barrier = _patched_drain_and_barrier

N_CORES = 8
B = 32768
B_CORE = B // N_CORES  # 4096
CHUNK = 512
N_CHUNKS = B_CORE // CHUNK  # 8
NT = 7  # pixel-group tiles of 4 image rows (112 pixels); 7*4 = 28 rows

F32 = mybir.dt.float32
BF16 = mybir.dt.bfloat16
RELU = mybir.ActivationFunctionType.Relu
IDENT = mybir.ActivationFunctionType.Identity

_NC_CACHE = {}


def _build_nc():
    nc = bass.Bass(
        "TRN2", target_bir_lowering=False, debug=False, num_devices=N_CORES
    )

    # chunk-major pixel-major input: row 784*n + 112*t + p, col = batch
    xh = nc.dram_tensor("xh", [N_CHUNKS * 784, CHUNK], F32, kind="ExternalInput")
    c1_d = nc.dram_tensor("c1", [112, 104], BF16, kind="ExternalInput")
    c2_d = nc.dram_tensor("c2", [112, 104], BF16, kind="ExternalInput")
    wp_d = nc.dram_tensor("wp", [104, 70], BF16, kind="ExternalInput")
    bias_d = nc.dram_tensor("bias_in", [10, 1], F32, kind="ExternalInput")
    out_t = nc.dram_tensor("out_t", [10, B_CORE], F32, kind="ExternalOutput")

    with TileContext(nc) as tc:
        with (
            tc.tile_pool(name="const", bufs=1) as cpool,
            tc.tile_pool(name="xc", bufs=1) as xpool,
            tc.tile_pool(name="ry_a", bufs=4) as rypool_a,
            tc.tile_pool(name="ry_v", bufs=4) as rypool_v,
            tc.tile_pool(name="outT", bufs=1) as opool,
            tc.tile_pool(name="yps_a", bufs=2, space="PSUM") as ypool_a,
            tc.tile_pool(name="yps_v", bufs=2, space="PSUM") as ypool_v,
            tc.tile_pool(name="warmp", bufs=1, space="PSUM") as warmpool,
            tc.tile_pool(name="opsum", bufs=2, space="PSUM") as opsum,
        ):
            def load_chunk(n):
                tile = xpool.tile([112, NT * CHUNK], BF16, tag=f"xc{n}")
                for lo, hi in ((0, 4), (4, 7)):
                    src = bass.AP(
                        xh,
                        (784 * n + 112 * lo) * CHUNK,
                        [[CHUNK, 112], [112 * CHUNK, hi - lo], [1, CHUNK]],
                    )
                    nc.gpsimd.dma_start(tile[:, CHUNK * lo : CHUNK * hi], src)
                return tile

            # Chunk 0's x load goes first on the SWDGE queue so compute can
            # start as early as possible; constants follow (needed only
            # slightly later by the warm-up touches).
            xc = [load_chunk(0)]
            c1_sb = cpool.tile([112, 104], BF16, tag="c1")
            nc.gpsimd.dma_start(c1_sb[:], c1_d.ap())
            c2_sb = cpool.tile([112, 104], BF16, tag="c2")
            nc.gpsimd.dma_start(c2_sb[:], c2_d.ap())
            wp_sb = cpool.tile([104, 70], BF16, tag="wp")
            nc.gpsimd.dma_start(wp_sb[:], wp_d.ap())
            bias_sb = cpool.tile([10, 1], F32, tag="bias")
            nc.gpsimd.dma_start(bias_sb[:], bias_d.ap())
            for n in range(1, N_CHUNKS):
                xc.append(load_chunk(n))

            outT_sb = opool.tile([10, B_CORE], F32, tag="outT")

            # PE HAM warm-up: the PE clock-gate only lifts to 2.4 GHz after
            # ~3.4us of sustained activity. Fill the initial DMA-wait window
            # with tiny matmuls on a junk tile so the real matmuls run warm.
            junk = cpool.tile([112, 8], BF16, tag="junk")
            nc.gpsimd.memset(junk[:], 0.0)
            warm = warmpool.tile([8, 8], F32, tag="warm")
            for _ in range(64):
                nc.tensor.matmul(warm[:], junk[:], junk[:])

            # Pre-touch each constant with a tiny op so real instructions'
            # dependency on its DMA is satisfied by engine program order
            # (walrus allows a single sync wait per instruction).
            nc.tensor.matmul(warm[0:4, 0:4], c1_sb[:, 0:4], c1_sb[:, 0:4])
            nc.tensor.matmul(warm[0:4, 0:4], c2_sb[:, 0:4], c2_sb[:, 0:4])
            nc.tensor.matmul(warm[0:4, 0:4], wp_sb[:, 0:4], wp_sb[:, 0:4])
            warm_act = cpool.tile([10, 1], F32, tag="warm_act")
            nc.scalar.activation(warm_act[:], bias_sb[:], IDENT, bias=bias_sb[:])

            for n in range(N_CHUNKS):
                # Touch matmul: absorbs this chunk's DMA wait on PE so the
                # conv matmuls only carry their PSUM-slot wait.
                nc.tensor.matmul(warm[0:4, 0:4], xc[n][:, 0:4], xc[n][:, 0:4])
                nc.tensor.matmul(
                    warm[0:4, 0:4],
                    xc[n][:, 6 * CHUNK : 6 * CHUNK + 4],
                    xc[n][:, 6 * CHUNK : 6 * CHUNK + 4],
                )
                rys = []
                for t in range(NT):
                    m = 104 if t < 6 else 52
                    on_act = t % 2 == 0
                    yps = (ypool_a if on_act else ypool_v).tile(
                        [m, CHUNK], F32, tag="yps"
                    )
                    nc.tensor.matmul(
                        yps[:],
                        c1_sb[:, 0:m],
                        xc[n][:, CHUNK * t : CHUNK * (t + 1)],
                        start=True,
                        stop=(t == 6),
                    )
                    if t < 6:
                        nc.tensor.matmul(
                            yps[:],
                            c2_sb[:],
                            xc[n][:, CHUNK * (t + 1) : CHUNK * (t + 2)],
                            start=False,
                            stop=True,
                        )
                    ry = (rypool_a if on_act else rypool_v).tile(
                        [m, CHUNK], BF16, tag="ry"
                    )
                    if on_act:
                        nc.scalar.activation(ry[:], yps[:], RELU)
                    else:
                        nc.vector.tensor_relu(ry[:], yps[:])
                    rys.append(ry)

                ops = opsum.tile([10, CHUNK], F32, tag="ops")
                for t in range(NT):
                    k = 104 if t < 6 else 52
                    nc.tensor.matmul(
                        ops[:],
                        wp_sb[0:k, 10 * t : 10 * (t + 1)],
                        rys[t][:],
                        start=(t == 0),
                        stop=(t == 6),
                    )
                nc.scalar.activation(
                    outT_sb[:, CHUNK * n : CHUNK * (n + 1)],
                    ops[:],
                    IDENT,
                    bias=bias_sb[:],
                )
                # SP queue carries no bulk loads, so all 8 per-chunk output
                # DMAs get fresh lanes and carry only the data wait; writing
                # per chunk hides the HBM write-receipt latency of all but
                # the last chunk behind compute.
                nc.sync.dma_start(
                    out_t.ap()[:, CHUNK * n : CHUNK * (n + 1)],
                    outT_sb[:, CHUNK * n : CHUNK * (n + 1)],
                )

    _strip_self_waits(nc)
    return nc


_ENGINE_SEM_PREFIX = {
    mybir.EngineType.PE: "PE_",
    mybir.EngineType.Activation: "Activation_",
    mybir.EngineType.DVE: "DVE_",
    mybir.EngineType.Pool: "Pool_",
    mybir.EngineType.SP: "SP_",
}


def _strip_self_waits(nc):
    """Drop semaphore waits an instruction holds on its OWN engine's
    completion counter. Engines execute their queue strictly in order, so
    a wait on the own-engine sem at a value covered by program order is
    redundant — but Tile still emits it, and walrus rejects compute
    instructions carrying more than one sync wait."""
    for fn in nc.m.functions:
        for blk in fn.blocks:
            for inst in blk.instructions:
                tn = type(inst).__name__
                if tn in ("InstDrain", "InstEventSemaphore", "InstDMACopy"):
                    continue
                si = inst.sync_info
                if si is None or not si.on_wait or len(si.on_wait) < 2:
                    continue
                pref = _ENGINE_SEM_PREFIX.get(inst.engine)
                if pref is None:
                    continue
                kept = [w for w in si.on_wait if not w.ant_name.startswith(pref)]
                if len(kept) != len(si.on_wait):
                    inst.sync_info = mybir.SyncInfo(
                        on_wait=kept, on_update=si.on_update
                    )


def _build_consts(conv_w, W, b):
    conv_w = np.asarray(conv_w, np.float32)
    W = np.asarray(W, np.float32)
    b = np.asarray(b, np.float32)

    # C1: input rows 4t+rl (rl 0..3) -> output conv rows 4t+il (il 0..3)
    # C2: input rows 4(t+1)+rl      -> output conv rows 4t+il
    c1 = np.zeros((112, 104), np.float32)
    c2 = np.zeros((112, 104), np.float32)
    for rl in range(4):
        for c in range(28):
            for il in range(4):
                for j in range(26):
                    dj = c - j
                    if not (0 <= dj <= 2):
                        continue
                    di = rl - il
                    if 0 <= di <= 2:
                        c1[rl * 28 + c, il * 26 + j] = conv_w[di, dj]
                    di2 = 4 + rl - il
                    if 0 <= di2 <= 2:
                        c2[rl * 28 + c, il * 26 + j] = conv_w[di2, dj]

    # W packed: block t holds rows for conv-output rows 4t..4t+3
    wp = np.zeros((104, 70), np.float32)
    for t in range(6):
        wp[:, 10 * t : 10 * (t + 1)] = W[104 * t : 104 * (t + 1)]
    wp[0:52, 60:70] = W[624:676]

    import ml_dtypes

    bf16 = ml_dtypes.bfloat16
    return (
        c1.astype(bf16),
        c2.astype(bf16),
        wp.astype(bf16),
        b.reshape(10, 1).copy(),
    )


def _run(inputs, trace=False):
    x = np.asarray(inputs["x"], np.float32)
    conv_w = inputs["conv_w"]
    W = inputs["W"]
    b = inputs["b"]

    if "nc" not in _NC_CACHE:
        _NC_CACHE["nc"] = _build_nc()
    nc = _NC_CACHE["nc"]

    c1, c2, wp, bias = _build_consts(conv_w, W, b)

    in_maps = []
    for c in range(N_CORES):
        shard = x[c * B_CORE : (c + 1) * B_CORE]  # [4096, 784]
        # [8, 512, 784] -> [8, 784, 512] -> rows (n, pixel), cols batch
        xh = np.ascontiguousarray(
            shard.reshape(N_CHUNKS, CHUNK, 784).transpose(0, 2, 1)
        ).reshape(N_CHUNKS * 784, CHUNK)
        in_maps.append(
            {"xh": xh, "c1": c1, "c2": c2, "wp": wp, "bias_in": bias}
        )

    res = run_bass_kernel_spmd(
        nc, in_maps, core_ids=list(range(N_CORES)), trace=trace
    )
    out = np.concatenate(
        [np.asarray(res.results[c]["out_t"]).T for c in range(N_CORES)], axis=0
    )
    return out, res


def kernel(**inputs) -> np.ndarray:
    return _run(inputs, trace=False)[0]
